# revision 16
# baseline (speedup 1.0000x reference)
"""Trainium2 Bass kernel for CombineRadialSpeciesWithAngularAdaptBasis.

Computation: for l in 0..5 (m = 2l+1):
    o_l = einsum('smp,pb->smb', values_l [N,m,P], W_l [P,B])   -> reshape (N*m, B)
    g_l = einsum('sxmp,pb->sxmb', grads_l [NG,3,m,P], W_l)     -> reshape (NG*3*m, B)
  output = concat([o_0, g_0, o_1, g_1, ... o_5, g_5], axis=0)

Strategy: data-parallel across samples on 8 NeuronCores; pure streaming GEMM
with tiny stationary weights -> HBM-DMA bound. All device I/O is bf16.

Per core the flat column space S=243000 is split into two halves A/B of
121500 columns processed in lockstep: chunk i loads x[:, c:c+n] (half A,
sync HWDGE queue) and x[:, HALF+c:...] (half B, scalar HWDGE queue); the PE
computes both 64-row results into one [128, n] PSUM tile (A -> partitions
0-63, B -> 64-127), DVE/ACT alternate downcast-copies per PSUM tile, and one
gpsimd (SWDGE) DMA writes the packed [128, n] bf16 chunk to y[128, 121500].
This packs the output across all 128 partitions (2x copy throughput vs a
[64, S] layout), halves the output-DMA count, and spreads the 70 MB/core of
HBM traffic across three DMA queues so no single queue serializes the
kernel (the previous layout's single 31 MB write queue was the critical
path at ~158 GB/s).
"""
import numpy as np
import ml_dtypes

BF16 = np.dtype(ml_dtypes.bfloat16)

N, NG, P, B, LMAX = 30000, 8000, 80, 64, 5
NCORES = 8
NV = N // NCORES      # 3750 values samples per core
NGV = NG // NCORES    # 1000 grads samples per core

NT = 512              # matmul moving-operand tile (one PSUM bank fp32)

# Region order matches the reference's output concatenation: v0,g0,v1,g1,...
REGIONS = []
for _l in range(LMAX + 1):
    _m = 2 * _l + 1
    REGIONS.append((False, _l, NV * _m))
    REGIONS.append((True, _l, NGV * 3 * _m))
STOT = sum(r[2] for r in REGIONS)  # 243000
HALF = STOT // 2                   # 121500

# Flat-column segments: (start_col, end_col, l)
SEGS = []
_off = 0
for _g, _l, _cols in REGIONS:
    SEGS.append((_off, _off + _cols, _l))
    _off += _cols

# chunk schedule over ONE half (both halves advance in lockstep): small
# chunks at the head fill the pipeline sooner; shrinking tail drains quickly
_SIZES = [2048, 4096] + [8192] * 13 + [4096, 2048, 1024, 512, 512, 668]
assert sum(_SIZES) == HALF
REARRANGE_READS = False
CHUNKS = []
_c = 0
for _sz in _SIZES:
    CHUNKS.append((_c, _sz))
    _c += _sz

_CACHE = {}


def _segments_in(lo, hi):
    """Yield (s, e, l) sub-intervals of [lo, hi) split at region bounds."""
    for s, e, l in SEGS:
        a, b = max(s, lo), min(e, hi)
        if a < b:
            yield a, b, l


def _build_program():
    """Build and finalize the (SPMD, per-core) Bass program once."""
    import concourse.bass as bass
    import concourse.tile as tile
    import concourse.mybir as mybir
    from concourse import bacc

    f32 = mybir.dt.float32
    bf16 = mybir.dt.bfloat16

    nc = bacc.Bacc("TRN2", target_bir_lowering=False, debug=False,
                   num_devices=NCORES, dynamic_dma_scratch_size=12288)
    x = nc.declare_dram_parameter("x", [P, STOT], bf16, isOutput=False)
    # all six W_l packed side by side -> one DMA
    wall = nc.declare_dram_parameter("wall", [P, (LMAX + 1) * B], bf16,
                                     isOutput=False)
    # packed output: rows 0-63 = half A columns, rows 64-127 = half B
    y = nc.declare_dram_parameter("y", [2 * B, HALF], bf16, isOutput=True)

    with tile.TileContext(nc) as tc:
        with (
            tc.tile_pool(name="wp", bufs=1) as wp,
            # 4 input bufs with PREFETCH=2: a read issued mid-chunk ci for
            # chunk ci+3 reuses chunk ci-1's buffer, whose matmuls are
            # already done -> the issuing engine never stalls and no
            # engine-order/PSUM-bank dependency cycle can form
            tc.tile_pool(name="inp", bufs=4) as inp,
            tc.tile_pool(name="outp", bufs=4) as outp,
            tc.tile_pool(name="psp", bufs=8, space="PSUM") as psp,
        ):
            wt = wp.tile([P, (LMAX + 1) * B], bf16, name="wt", tag="wt")
            nc.scalar.dma_start(wt[:], wall[:, :])
            w_sb = [wt[:, l * B:(l + 1) * B] for l in range(LMAX + 1)]

            # issue the input DMAs PREFETCH chunks ahead of compute so the
            # scalar engine's read issuance never sits behind its share of
            # the PSUM->SBUF copies in program order (HWDGE rings drain
            # asynchronously once the instruction issues)
            PREFETCH = 2
            xts = {}

            def issue_read(ci):
                c0, csz = CHUNKS[ci]
                xa = inp.tile([P, csz], bf16, name=f"xa_{ci}", tag="xa")
                xb = inp.tile([P, csz], bf16, name=f"xb_{ci}", tag="xb")
                # The SDMA read path pipelines descriptors <=~4KB at line
                # rate (~26 GB/s/engine) but serializes the HBM round trip
                # for >=8KB ones (~14-17 GB/s). Emit 4KB read descriptors in
                # k-outer order so consecutive ring descriptors touch
                # different partitions and cannot be re-aggregated into big
                # packets. (Writes are posted and prefer big descriptors.)
                def rd(dst, src):
                    if REARRANGE_READS and csz % 2048 == 0:
                        dst = dst.rearrange("p (k d) -> k p d", d=2048)
                        src = src.rearrange("p (k d) -> k p d", d=2048)
                    return dst, src

                nc.sync.dma_start(*rd(xa[:], x[:, c0:c0 + csz]))
                nc.scalar.dma_start(
                    *rd(xb[:], x[:, HALF + c0:HALF + c0 + csz]))
                xts[ci] = (xa, xb)

            for ci in range(min(PREFETCH + 1, len(CHUNKS))):
                issue_read(ci)

            for ci, (c0, csz) in enumerate(CHUNKS):
                xa, xb = xts.pop(ci)
                ot = outp.tile([2 * B, csz], bf16, name=f"ot_{ci}", tag="ot")
                for ti, k0 in enumerate(range(0, csz, NT)):
                    n = min(NT, csz - k0)
                    ps = psp.tile([2 * B, n], f32, name=f"ps_{ci}_{k0}",
                                  tag="ps")
                    for sa, sb, l in _segments_in(c0 + k0, c0 + k0 + n):
                        ra, rb = sa - c0, sb - c0
                        nc.tensor.matmul(ps[0:B, ra - k0:rb - k0],
                                         lhsT=w_sb[l],
                                         rhs=xa[:, ra:rb],
                                         start=True, stop=True)
                    for sa, sb, l in _segments_in(HALF + c0 + k0,
                                                  HALF + c0 + k0 + n):
                        ra, rb = sa - HALF - c0, sb - HALF - c0
                        nc.tensor.matmul(ps[B:2 * B, ra - k0:rb - k0],
                                         lhsT=w_sb[l],
                                         rhs=xb[:, ra:rb],
                                         start=True, stop=True)
                    # DVE takes 2/3 of the downcast copies, ACT 1/3 (ACT
                    # also issues the xb reads; keep it lightly loaded)
                    if ti % 3 != 2:
                        nc.vector.tensor_copy(ot[:, k0:k0 + n], ps[:])
                    else:
                        nc.scalar.copy(ot[:, k0:k0 + n], ps[:])
                    # interleave next chunk's reads mid-compute so their
                    # issue point sits between copies, not after them
                    if ti == 1 and ci + PREFETCH + 1 < len(CHUNKS):
                        issue_read(ci + PREFETCH + 1)
                # writes are line-rate at any descriptor size; one SWDGE
                # queue handles all 31.1MB comfortably
                nc.gpsimd.dma_start(y[:, c0:c0 + csz], ot[:])

    nc.finalize()
    return nc


def _get_program():
    if "nc" not in _CACHE:
        _CACHE["nc"] = _build_program()
    return _CACHE["nc"]


def _register_ntff_hook():
    """antenv.axon_hooks is absent in this image; the .so supports NTFF
    profiling — install the shim so run_bass_kernel_spmd(trace=True) works."""
    import sys, types
    try:
        from antenv.axon_hooks import get_axon_ntff_profile_hook  # noqa: F401
        return
    except ImportError:
        pass
    import antenv
    from trn_agent_boot.trn_boot import _ntff_profile_via_ctypes
    mod = types.ModuleType("antenv.axon_hooks")
    mod._hook = _ntff_profile_via_ctypes('/opt/axon/libaxon_pjrt.so')
    mod.get_axon_ntff_profile_hook = lambda: mod._hook
    mod.set_axon_ntff_profile_hook = lambda h: setattr(mod, '_hook', h)
    sys.modules["antenv.axon_hooks"] = mod
    antenv.axon_hooks = mod


LAST_EXEC_TIME_NS = None
LAST_MEAN_EXEC_TIME_NS = None


def kernel(trace=False, trace_all_cores=False, **inputs):
    global LAST_EXEC_TIME_NS, LAST_MEAN_EXEC_TIME_NS
    from concourse.bass_utils import run_bass_kernel_spmd

    # ---- host-side: shard, transpose to [P, S], pack flat, cast bf16 ----
    wall = np.concatenate(
        [np.asarray(inputs[f"W_l{l}"]) for l in range(LMAX + 1)],
        axis=1).astype(BF16)
    in_maps = [{"x": np.empty((P, STOT), dtype=BF16), "wall": wall}
               for _ in range(NCORES)]
    off = 0
    for g, l, cols in REGIONS:
        src = inputs[f"grads_l{l}"] if g else inputs[f"values_l{l}"]
        src = np.asarray(src)
        ns = NGV if g else NV
        for i in range(NCORES):
            blk = src[i * ns:(i + 1) * ns].reshape(cols, P).astype(BF16)
            in_maps[i]["x"][:, off:off + cols] = blk.T
        off += cols

    nc = _get_program()
    kwargs = {}
    if trace:
        _register_ntff_hook()
        kwargs["trace"] = True
        if trace_all_cores:
            kwargs["trace_cores"] = list(range(NCORES))
    res = run_bass_kernel_spmd(nc, in_maps, list(range(NCORES)), **kwargs)
    LAST_EXEC_TIME_NS = res.exec_time_ns
    LAST_MEAN_EXEC_TIME_NS = res.mean_exec_time_ns

    # ---- gather: split flat intervals across the packed halves, upcast ----
    outs = [res.results[i]["y"] for i in range(NCORES)]
    total_rows = NCORES * STOT
    final = np.empty((total_rows, B), dtype=np.float32)
    row = 0
    off = 0
    for g, l, cols in REGIONS:
        for i in range(NCORES):
            s, e = off, off + cols
            r = row
            if s < HALF:
                e0 = min(e, HALF)
                final[r:r + (e0 - s)] = outs[i][0:B, s:e0].T
                r += e0 - s
                s = e0
            if s < e:
                final[r:r + (e - s)] = outs[i][B:2 * B, s - HALF:e - HALF].T
            row += cols
        off += cols
    return final


# revision 20
# speedup vs baseline: 1.0552x; 1.0552x over previous
"""Trainium2 Bass kernel for CombineRadialSpeciesWithAngularAdaptBasis.

Computation: for l in 0..5 (m = 2l+1):
    o_l = einsum('smp,pb->smb', values_l [N,m,P], W_l [P,B])   -> reshape (N*m, B)
    g_l = einsum('sxmp,pb->sxmb', grads_l [NG,3,m,P], W_l)     -> reshape (NG*3*m, B)
  output = concat([o_0, g_0, o_1, g_1, ... o_5, g_5], axis=0)

Strategy: data-parallel across samples on 8 NeuronCores; pure streaming GEMM
with tiny stationary weights -> HBM-DMA bound. All device I/O is bf16.

Per core the flat column space S=243000 is split into two halves A/B of
121500 columns processed in lockstep: chunk i loads x[:, c:c+n] (half A,
sync HWDGE queue) and x[:, HALF+c:...] (half B, scalar HWDGE queue); the PE
computes both 64-row results into one [128, n] PSUM tile (A -> partitions
0-63, B -> 64-127), DVE/ACT alternate downcast-copies per PSUM tile, and one
gpsimd (SWDGE) DMA writes the packed [128, n] bf16 chunk to y[128, 121500].
This packs the output across all 128 partitions (2x copy throughput vs a
[64, S] layout), halves the output-DMA count, and spreads the 70 MB/core of
HBM traffic across three DMA queues so no single queue serializes the
kernel (the previous layout's single 31 MB write queue was the critical
path at ~158 GB/s).
"""
import numpy as np
import ml_dtypes

BF16 = np.dtype(ml_dtypes.bfloat16)

N, NG, P, B, LMAX = 30000, 8000, 80, 64, 5
NCORES = 8
NV = N // NCORES      # 3750 values samples per core
NGV = NG // NCORES    # 1000 grads samples per core

NT = 512              # matmul moving-operand tile (one PSUM bank fp32)

# Region order matches the reference's output concatenation: v0,g0,v1,g1,...
REGIONS = []
for _l in range(LMAX + 1):
    _m = 2 * _l + 1
    REGIONS.append((False, _l, NV * _m))
    REGIONS.append((True, _l, NGV * 3 * _m))
STOT = sum(r[2] for r in REGIONS)  # 243000
HALF = STOT // 2                   # 121500

# Flat-column segments: (start_col, end_col, l)
SEGS = []
_off = 0
for _g, _l, _cols in REGIONS:
    SEGS.append((_off, _off + _cols, _l))
    _off += _cols

# chunk schedule over ONE half (both halves advance in lockstep): small
# chunks at the head fill the pipeline sooner; shrinking tail drains quickly
_SIZES = [2048, 4096] + [8192] * 13 + [4096, 2048, 1024, 512, 512, 668]
assert sum(_SIZES) == HALF

# x is packed in DRAM with a PAD-column gap after every BLK-column block
# (per half): DRAM-side discontinuity between consecutive descriptors keeps
# the SDMA from re-aggregating 4KB read descriptors into 16KB packets (the
# read path pipelines <=4KB descriptors at ~26 GB/s/engine vs ~16 GB/s for
# 16KB ones).
BLK = 2048
PAD = 16
HALFP = (HALF // BLK) * (BLK + PAD) + (HALF % BLK)  # padded cols per half


def _pmap(c):
    """half-local flat column -> padded column"""
    return (c // BLK) * (BLK + PAD) + (c % BLK)
CHUNKS = []
_c = 0
for _sz in _SIZES:
    CHUNKS.append((_c, _sz))
    _c += _sz

_CACHE = {}


def _segments_in(lo, hi):
    """Yield (s, e, l) sub-intervals of [lo, hi) split at region bounds."""
    for s, e, l in SEGS:
        a, b = max(s, lo), min(e, hi)
        if a < b:
            yield a, b, l


def _build_program():
    """Build and finalize the (SPMD, per-core) Bass program once."""
    import concourse.bass as bass
    import concourse.tile as tile
    import concourse.mybir as mybir
    from concourse import bacc

    f32 = mybir.dt.float32
    bf16 = mybir.dt.bfloat16

    nc = bacc.Bacc("TRN2", target_bir_lowering=False, debug=False,
                   num_devices=NCORES, dynamic_dma_scratch_size=12288)
    x = nc.declare_dram_parameter("x", [P, 2 * HALFP], bf16, isOutput=False)
    # all six W_l packed side by side -> one DMA
    wall = nc.declare_dram_parameter("wall", [P, (LMAX + 1) * B], bf16,
                                     isOutput=False)
    # packed output: rows 0-63 = half A columns, rows 64-127 = half B
    y = nc.declare_dram_parameter("y", [2 * B, HALF], bf16, isOutput=True)

    with tile.TileContext(nc) as tc:
        with (
            tc.tile_pool(name="wp", bufs=1) as wp,
            # 4 input bufs with PREFETCH=2: a read issued mid-chunk ci for
            # chunk ci+3 reuses chunk ci-1's buffer, whose matmuls are
            # already done -> the issuing engine never stalls and no
            # engine-order/PSUM-bank dependency cycle can form
            tc.tile_pool(name="inp", bufs=4) as inp,
            tc.tile_pool(name="outp", bufs=4) as outp,
            tc.tile_pool(name="psp", bufs=8, space="PSUM") as psp,
        ):
            wt = wp.tile([P, (LMAX + 1) * B], bf16, name="wt", tag="wt")
            nc.scalar.dma_start(wt[:], wall[:, :])
            w_sb = [wt[:, l * B:(l + 1) * B] for l in range(LMAX + 1)]

            # issue the input DMAs PREFETCH chunks ahead of compute so the
            # scalar engine's read issuance never sits behind its share of
            # the PSUM->SBUF copies in program order (HWDGE rings drain
            # asynchronously once the instruction issues)
            PREFETCH = 2
            xts = {}

            def issue_read(ci):
                c0, csz = CHUNKS[ci]
                xa = inp.tile([P, csz], bf16, name=f"xa_{ci}", tag="xa")
                xb = inp.tile([P, csz], bf16, name=f"xb_{ci}", tag="xb")

                def rd(eng, dst, base):
                    p0 = base + _pmap(c0)
                    if csz % BLK != 0:
                        # small chunk: sits inside one padded block
                        eng.dma_start(dst[:], x[:, p0:p0 + csz])
                    elif ci % 2 == 0:
                        # style D: one DMA, 3-dim AP skipping the pads
                        k = csz // BLK
                        src = x[:, p0:p0 + k * (BLK + PAD)].rearrange(
                            "p (k d) -> p k d", d=BLK + PAD)[:, :, 0:BLK]
                        eng.dma_start(
                            dst[:].rearrange("p (k d) -> p k d", d=BLK), src)
                    else:
                        # style B: one sub-DMA per 2048-col block
                        for j in range(csz // BLK):
                            eng.dma_start(
                                dst[:, j * BLK:(j + 1) * BLK],
                                x[:, p0 + j * (BLK + PAD):
                                     p0 + j * (BLK + PAD) + BLK])

                rd(nc.sync, xa, 0)
                rd(nc.scalar, xb, HALFP)
                xts[ci] = (xa, xb)

            for ci in range(min(PREFETCH + 1, len(CHUNKS))):
                issue_read(ci)

            for ci, (c0, csz) in enumerate(CHUNKS):
                xa, xb = xts.pop(ci)
                ot = outp.tile([2 * B, csz], bf16, name=f"ot_{ci}", tag="ot")
                for ti, k0 in enumerate(range(0, csz, NT)):
                    n = min(NT, csz - k0)
                    ps = psp.tile([2 * B, n], f32, name=f"ps_{ci}_{k0}",
                                  tag="ps")
                    for sa, sb, l in _segments_in(c0 + k0, c0 + k0 + n):
                        ra, rb = sa - c0, sb - c0
                        nc.tensor.matmul(ps[0:B, ra - k0:rb - k0],
                                         lhsT=w_sb[l],
                                         rhs=xa[:, ra:rb],
                                         start=True, stop=True)
                    for sa, sb, l in _segments_in(HALF + c0 + k0,
                                                  HALF + c0 + k0 + n):
                        ra, rb = sa - HALF - c0, sb - HALF - c0
                        nc.tensor.matmul(ps[B:2 * B, ra - k0:rb - k0],
                                         lhsT=w_sb[l],
                                         rhs=xb[:, ra:rb],
                                         start=True, stop=True)
                    # DVE takes 2/3 of the downcast copies, ACT 1/3 (ACT
                    # also issues the xb reads; keep it lightly loaded)
                    if ti % 3 != 2:
                        nc.vector.tensor_copy(ot[:, k0:k0 + n], ps[:])
                    else:
                        nc.scalar.copy(ot[:, k0:k0 + n], ps[:])
                    # interleave next chunk's reads mid-compute so their
                    # issue point sits between copies, not after them
                    if ti == 1 and ci + PREFETCH + 1 < len(CHUNKS):
                        issue_read(ci + PREFETCH + 1)
                # writes are line-rate at any descriptor size; one SWDGE
                # queue handles all 31.1MB comfortably
                nc.gpsimd.dma_start(y[:, c0:c0 + csz], ot[:])

    nc.finalize()
    return nc


def _get_program():
    if "nc" not in _CACHE:
        _CACHE["nc"] = _build_program()
    return _CACHE["nc"]


def _register_ntff_hook():
    """antenv.axon_hooks is absent in this image; the .so supports NTFF
    profiling — install the shim so run_bass_kernel_spmd(trace=True) works."""
    import sys, types
    try:
        from antenv.axon_hooks import get_axon_ntff_profile_hook  # noqa: F401
        return
    except ImportError:
        pass
    import antenv
    from trn_agent_boot.trn_boot import _ntff_profile_via_ctypes
    mod = types.ModuleType("antenv.axon_hooks")
    mod._hook = _ntff_profile_via_ctypes('/opt/axon/libaxon_pjrt.so')
    mod.get_axon_ntff_profile_hook = lambda: mod._hook
    mod.set_axon_ntff_profile_hook = lambda h: setattr(mod, '_hook', h)
    sys.modules["antenv.axon_hooks"] = mod
    antenv.axon_hooks = mod


LAST_EXEC_TIME_NS = None
LAST_MEAN_EXEC_TIME_NS = None


def kernel(trace=False, trace_all_cores=False, **inputs):
    global LAST_EXEC_TIME_NS, LAST_MEAN_EXEC_TIME_NS
    from concourse.bass_utils import run_bass_kernel_spmd

    # ---- host-side: shard, transpose to [P, S], pack flat, cast bf16 ----
    wall = np.concatenate(
        [np.asarray(inputs[f"W_l{l}"]) for l in range(LMAX + 1)],
        axis=1).astype(BF16)
    flats = [np.empty((P, STOT), dtype=BF16) for _ in range(NCORES)]
    off = 0
    for g, l, cols in REGIONS:
        src = inputs[f"grads_l{l}"] if g else inputs[f"values_l{l}"]
        src = np.asarray(src)
        ns = NGV if g else NV
        for i in range(NCORES):
            blk = src[i * ns:(i + 1) * ns].reshape(cols, P).astype(BF16)
            flats[i][:, off:off + cols] = blk.T
        off += cols
    # scatter each half's 2048-col blocks into the PAD-gapped DRAM layout
    nfull = HALF // BLK
    in_maps = []
    for i in range(NCORES):
        xp = np.zeros((P, 2 * HALFP), dtype=BF16)
        for h in range(2):
            fl = flats[i][:, h * HALF:(h + 1) * HALF]
            dst = xp[:, h * HALFP:(h + 1) * HALFP]
            dst[:, :nfull * (BLK + PAD)].reshape(P, nfull, BLK + PAD)[
                :, :, :BLK] = fl[:, :nfull * BLK].reshape(P, nfull, BLK)
            dst[:, nfull * (BLK + PAD):] = fl[:, nfull * BLK:]
        in_maps.append({"x": xp, "wall": wall})

    nc = _get_program()
    kwargs = {}
    if trace:
        _register_ntff_hook()
        kwargs["trace"] = True
        if trace_all_cores:
            kwargs["trace_cores"] = list(range(NCORES))
    res = run_bass_kernel_spmd(nc, in_maps, list(range(NCORES)), **kwargs)
    LAST_EXEC_TIME_NS = res.exec_time_ns
    LAST_MEAN_EXEC_TIME_NS = res.mean_exec_time_ns

    # ---- gather: split flat intervals across the packed halves, upcast ----
    outs = [res.results[i]["y"] for i in range(NCORES)]
    total_rows = NCORES * STOT
    final = np.empty((total_rows, B), dtype=np.float32)
    row = 0
    off = 0
    for g, l, cols in REGIONS:
        for i in range(NCORES):
            s, e = off, off + cols
            r = row
            if s < HALF:
                e0 = min(e, HALF)
                final[r:r + (e0 - s)] = outs[i][0:B, s:e0].T
                r += e0 - s
                s = e0
            if s < e:
                final[r:r + (e - s)] = outs[i][B:2 * B, s - HALF:e - HALF].T
            row += cols
        off += cols
    return final


# revision 24
# speedup vs baseline: 1.0601x; 1.0047x over previous
"""Trainium2 Bass kernel for CombineRadialSpeciesWithAngularAdaptBasis.

Computation: for l in 0..5 (m = 2l+1):
    o_l = einsum('smp,pb->smb', values_l [N,m,P], W_l [P,B])   -> reshape (N*m, B)
    g_l = einsum('sxmp,pb->sxmb', grads_l [NG,3,m,P], W_l)     -> reshape (NG*3*m, B)
  output = concat([o_0, g_0, o_1, g_1, ... o_5, g_5], axis=0)

Strategy: data-parallel across samples on 8 NeuronCores; pure streaming GEMM
with tiny stationary weights -> HBM-DMA bound. All device I/O is bf16.

Per core the flat column space S=243000 is split into two halves A/B of
121500 columns processed in lockstep: chunk i loads x[:, c:c+n] (half A,
sync HWDGE queue) and x[:, HALF+c:...] (half B, scalar HWDGE queue); the PE
computes both 64-row results into one [128, n] PSUM tile (A -> partitions
0-63, B -> 64-127), DVE/ACT alternate downcast-copies per PSUM tile, and one
gpsimd (SWDGE) DMA writes the packed [128, n] bf16 chunk to y[128, 121500].
This packs the output across all 128 partitions (2x copy throughput vs a
[64, S] layout), halves the output-DMA count, and spreads the 70 MB/core of
HBM traffic across three DMA queues so no single queue serializes the
kernel (the previous layout's single 31 MB write queue was the critical
path at ~158 GB/s).
"""
import numpy as np
import ml_dtypes

BF16 = np.dtype(ml_dtypes.bfloat16)

N, NG, P, B, LMAX = 30000, 8000, 80, 64, 5
NCORES = 8
NV = N // NCORES      # 3750 values samples per core
NGV = NG // NCORES    # 1000 grads samples per core

NT = 512              # matmul moving-operand tile (one PSUM bank fp32)

# Region order matches the reference's output concatenation: v0,g0,v1,g1,...
REGIONS = []
for _l in range(LMAX + 1):
    _m = 2 * _l + 1
    REGIONS.append((False, _l, NV * _m))
    REGIONS.append((True, _l, NGV * 3 * _m))
STOT = sum(r[2] for r in REGIONS)  # 243000
HALF = STOT // 2                   # 121500

# Flat-column segments: (start_col, end_col, l)
SEGS = []
_off = 0
for _g, _l, _cols in REGIONS:
    SEGS.append((_off, _off + _cols, _l))
    _off += _cols

# chunk schedule over ONE half (both halves advance in lockstep): small
# chunks at the head fill the pipeline sooner; shrinking tail drains quickly
_SIZES = [2048, 2048] + [4096] * 28 + [1024, 1024, 668]
assert sum(_SIZES) == HALF

# x is packed in DRAM with a PAD-column gap after every BLK-column block
# (per half): DRAM-side discontinuity between consecutive descriptors keeps
# the SDMA from re-aggregating 4KB read descriptors into 16KB packets (the
# read path pipelines <=4KB descriptors at ~26 GB/s/engine vs ~16 GB/s for
# 16KB ones).
BLK = 2048
PAD = 16
HALFP = (HALF // BLK) * (BLK + PAD) + (HALF % BLK)  # padded cols per half


def _pmap(c):
    """half-local flat column -> padded column"""
    return (c // BLK) * (BLK + PAD) + (c % BLK)
CHUNKS = []
_c = 0
for _sz in _SIZES:
    CHUNKS.append((_c, _sz))
    _c += _sz

# writes are issued once per PAIR of chunks so write descriptors stay at
# 16KB/partition (the posted-write line-rate size) while reads use the
# small-chunk granularity that keeps their descriptors at 4KB
PAIRS = [(i, i + 1) for i in range(0, len(CHUNKS) - 1, 2)]
if len(CHUNKS) % 2 == 1:
    PAIRS.append((len(CHUNKS) - 1,))

_CACHE = {}


def _segments_in(lo, hi):
    """Yield (s, e, l) sub-intervals of [lo, hi) split at region bounds."""
    for s, e, l in SEGS:
        a, b = max(s, lo), min(e, hi)
        if a < b:
            yield a, b, l


def _build_program():
    """Build and finalize the (SPMD, per-core) Bass program once."""
    import concourse.bass as bass
    import concourse.tile as tile
    import concourse.mybir as mybir
    from concourse import bacc

    f32 = mybir.dt.float32
    bf16 = mybir.dt.bfloat16

    nc = bacc.Bacc("TRN2", target_bir_lowering=False, debug=False,
                   num_devices=NCORES, dynamic_dma_scratch_size=12288)
    x = nc.declare_dram_parameter("x", [P, 2 * HALFP], bf16, isOutput=False)
    # all six W_l packed side by side -> one DMA
    wall = nc.declare_dram_parameter("wall", [P, (LMAX + 1) * B], bf16,
                                     isOutput=False)
    # packed output: rows 0-63 = half A columns, rows 64-127 = half B
    y = nc.declare_dram_parameter("y", [2 * B, HALF], bf16, isOutput=True)

    with tile.TileContext(nc) as tc:
        with (
            tc.tile_pool(name="wp", bufs=1) as wp,
            # 8 input bufs with PREFETCH=5: a read issued mid-chunk ci for
            # chunk ci+6 reuses chunk ci-2's buffer, whose matmuls are long
            # done -> the issuing engine never stalls and no
            # engine-order/PSUM-bank dependency cycle can form; the deep
            # read-ahead keeps both HWDGE queues busy continuously
            tc.tile_pool(name="inp", bufs=8) as inp,
            tc.tile_pool(name="outp", bufs=3) as outp,
            tc.tile_pool(name="psp", bufs=8, space="PSUM") as psp,
        ):
            wt = wp.tile([P, (LMAX + 1) * B], bf16, name="wt", tag="wt")
            nc.scalar.dma_start(wt[:], wall[:, :])
            w_sb = [wt[:, l * B:(l + 1) * B] for l in range(LMAX + 1)]

            # issue the input DMAs PREFETCH chunks ahead of compute so read
            # issuance never sits behind copies in engine program order
            # (HWDGE rings drain asynchronously once the instruction issues)
            PREFETCH = 5
            xts = {}

            def issue_read(ci):
                c0, csz = CHUNKS[ci]
                xa = inp.tile([P, csz], bf16, name=f"xa_{ci}", tag="xa")
                xb = inp.tile([P, csz], bf16, name=f"xb_{ci}", tag="xb")

                def rd(eng, dst, base):
                    p0 = base + _pmap(c0)
                    if csz % BLK != 0:
                        # small chunk: sits inside one padded block
                        eng.dma_start(dst[:], x[:, p0:p0 + csz])
                    else:
                        # one DMA, 3-dim AP skipping the pads: 4KB read
                        # descriptors that cannot re-aggregate
                        k = csz // BLK
                        src = x[:, p0:p0 + k * (BLK + PAD)].rearrange(
                            "p (k d) -> p k d", d=BLK + PAD)[:, :, 0:BLK]
                        eng.dma_start(
                            dst[:].rearrange("p (k d) -> p k d", d=BLK), src)

                rd(nc.sync, xa, 0)
                rd(nc.scalar, xb, HALFP)
                xts[ci] = (xa, xb)

            for ci in range(min(PREFETCH + 1, len(CHUNKS))):
                issue_read(ci)

            for pi, pair in enumerate(PAIRS):
                pc0 = CHUNKS[pair[0]][0]
                pcs = sum(CHUNKS[ci][1] for ci in pair)
                ot = outp.tile([2 * B, pcs], bf16, name=f"ot_{pi}", tag="ot")
                poff = 0
                for ci in pair:
                    c0, csz = CHUNKS[ci]
                    xa, xb = xts.pop(ci)
                    for ti, k0 in enumerate(range(0, csz, NT)):
                        n = min(NT, csz - k0)
                        ps = psp.tile([2 * B, n], f32, name=f"ps_{ci}_{k0}",
                                      tag="ps")
                        for sa, sb, l in _segments_in(c0 + k0, c0 + k0 + n):
                            ra, rb = sa - c0, sb - c0
                            nc.tensor.matmul(ps[0:B, ra - k0:rb - k0],
                                             lhsT=w_sb[l],
                                             rhs=xa[:, ra:rb],
                                             start=True, stop=True)
                        for sa, sb, l in _segments_in(HALF + c0 + k0,
                                                      HALF + c0 + k0 + n):
                            ra, rb = sa - HALF - c0, sb - HALF - c0
                            nc.tensor.matmul(ps[B:2 * B, ra - k0:rb - k0],
                                             lhsT=w_sb[l],
                                             rhs=xb[:, ra:rb],
                                             start=True, stop=True)
                        # DVE takes 2/3 of the downcast copies, ACT 1/3
                        # (ACT also issues the xb reads)
                        if ti % 3 != 2:
                            nc.vector.tensor_copy(
                                ot[:, poff + k0:poff + k0 + n], ps[:])
                        else:
                            nc.scalar.copy(
                                ot[:, poff + k0:poff + k0 + n], ps[:])
                        # interleave next reads mid-compute so their issue
                        # point sits between copies, not after them
                        if ti == 1 and ci + PREFETCH + 1 < len(CHUNKS):
                            issue_read(ci + PREFETCH + 1)
                    poff += csz
                # one write per pair: 16KB/partition descriptors (posted
                # writes run at line rate only at large descriptor size)
                nc.gpsimd.dma_start(y[:, pc0:pc0 + pcs], ot[:])

    nc.finalize()
    return nc


def _get_program():
    if "nc" not in _CACHE:
        _CACHE["nc"] = _build_program()
    return _CACHE["nc"]


def _register_ntff_hook():
    """antenv.axon_hooks is absent in this image; the .so supports NTFF
    profiling — install the shim so run_bass_kernel_spmd(trace=True) works."""
    import sys, types
    try:
        from antenv.axon_hooks import get_axon_ntff_profile_hook  # noqa: F401
        return
    except ImportError:
        pass
    import antenv
    from trn_agent_boot.trn_boot import _ntff_profile_via_ctypes
    mod = types.ModuleType("antenv.axon_hooks")
    mod._hook = _ntff_profile_via_ctypes('/opt/axon/libaxon_pjrt.so')
    mod.get_axon_ntff_profile_hook = lambda: mod._hook
    mod.set_axon_ntff_profile_hook = lambda h: setattr(mod, '_hook', h)
    sys.modules["antenv.axon_hooks"] = mod
    antenv.axon_hooks = mod


LAST_EXEC_TIME_NS = None
LAST_MEAN_EXEC_TIME_NS = None


def kernel(trace=False, trace_all_cores=False, **inputs):
    global LAST_EXEC_TIME_NS, LAST_MEAN_EXEC_TIME_NS
    from concourse.bass_utils import run_bass_kernel_spmd

    # ---- host-side: shard, transpose to [P, S], pack flat, cast bf16 ----
    wall = np.concatenate(
        [np.asarray(inputs[f"W_l{l}"]) for l in range(LMAX + 1)],
        axis=1).astype(BF16)
    flats = [np.empty((P, STOT), dtype=BF16) for _ in range(NCORES)]
    off = 0
    for g, l, cols in REGIONS:
        src = inputs[f"grads_l{l}"] if g else inputs[f"values_l{l}"]
        src = np.asarray(src)
        ns = NGV if g else NV
        for i in range(NCORES):
            blk = src[i * ns:(i + 1) * ns].reshape(cols, P).astype(BF16)
            flats[i][:, off:off + cols] = blk.T
        off += cols
    # scatter each half's 2048-col blocks into the PAD-gapped DRAM layout
    nfull = HALF // BLK
    in_maps = []
    for i in range(NCORES):
        xp = np.zeros((P, 2 * HALFP), dtype=BF16)
        for h in range(2):
            fl = flats[i][:, h * HALF:(h + 1) * HALF]
            dst = xp[:, h * HALFP:(h + 1) * HALFP]
            dst[:, :nfull * (BLK + PAD)].reshape(P, nfull, BLK + PAD)[
                :, :, :BLK] = fl[:, :nfull * BLK].reshape(P, nfull, BLK)
            dst[:, nfull * (BLK + PAD):] = fl[:, nfull * BLK:]
        in_maps.append({"x": xp, "wall": wall})

    nc = _get_program()
    kwargs = {}
    if trace:
        _register_ntff_hook()
        kwargs["trace"] = True
        if trace_all_cores:
            kwargs["trace_cores"] = list(range(NCORES))
    res = run_bass_kernel_spmd(nc, in_maps, list(range(NCORES)), **kwargs)
    LAST_EXEC_TIME_NS = res.exec_time_ns
    LAST_MEAN_EXEC_TIME_NS = res.mean_exec_time_ns

    # ---- gather: split flat intervals across the packed halves, upcast ----
    outs = [res.results[i]["y"] for i in range(NCORES)]
    total_rows = NCORES * STOT
    final = np.empty((total_rows, B), dtype=np.float32)
    row = 0
    off = 0
    for g, l, cols in REGIONS:
        for i in range(NCORES):
            s, e = off, off + cols
            r = row
            if s < HALF:
                e0 = min(e, HALF)
                final[r:r + (e0 - s)] = outs[i][0:B, s:e0].T
                r += e0 - s
                s = e0
            if s < e:
                final[r:r + (e - s)] = outs[i][B:2 * B, s - HALF:e - HALF].T
            row += cols
        off += cols
    return final


# revision 28
# speedup vs baseline: 1.1260x; 1.0621x over previous
"""Trainium2 Bass kernel for CombineRadialSpeciesWithAngularAdaptBasis.

Computation: for l in 0..5 (m = 2l+1):
    o_l = einsum('smp,pb->smb', values_l [N,m,P], W_l [P,B])   -> reshape (N*m, B)
    g_l = einsum('sxmp,pb->sxmb', grads_l [NG,3,m,P], W_l)     -> reshape (NG*3*m, B)
  output = concat([o_0, g_0, o_1, g_1, ... o_5, g_5], axis=0)

Strategy: data-parallel across samples on 8 NeuronCores; pure streaming GEMM
with tiny stationary weights -> HBM-DMA bound. All device I/O is bf16.

Per core the flat column space S=243000 is split into two halves A/B of
121500 columns processed in lockstep: chunk i loads x[:, c:c+n] (half A,
sync HWDGE queue) and x[:, HALF+c:...] (half B, scalar HWDGE queue); the PE
computes both 64-row results into one [128, n] PSUM tile (A -> partitions
0-63, B -> 64-127), DVE/ACT alternate downcast-copies per PSUM tile, and one
gpsimd (SWDGE) DMA writes the packed [128, n] bf16 chunk to y[128, 121500].
This packs the output across all 128 partitions (2x copy throughput vs a
[64, S] layout), halves the output-DMA count, and spreads the 70 MB/core of
HBM traffic across three DMA queues so no single queue serializes the
kernel (the previous layout's single 31 MB write queue was the critical
path at ~158 GB/s).
"""
import numpy as np
import ml_dtypes

BF16 = np.dtype(ml_dtypes.bfloat16)

N, NG, P, B, LMAX = 30000, 8000, 80, 64, 5
NCORES = 8
NV = N // NCORES      # 3750 values samples per core
NGV = NG // NCORES    # 1000 grads samples per core

NT = 512              # matmul moving-operand tile (one PSUM bank fp32)

# Region order matches the reference's output concatenation: v0,g0,v1,g1,...
REGIONS = []
for _l in range(LMAX + 1):
    _m = 2 * _l + 1
    REGIONS.append((False, _l, NV * _m))
    REGIONS.append((True, _l, NGV * 3 * _m))
STOT = sum(r[2] for r in REGIONS)  # 243000
HALF = STOT // 2                   # 121500

# Flat-column segments: (start_col, end_col, l)
SEGS = []
_off = 0
for _g, _l, _cols in REGIONS:
    SEGS.append((_off, _off + _cols, _l))
    _off += _cols

# chunk schedule over ONE half (both halves advance in lockstep): small
# chunks at the head fill the pipeline sooner; shrinking tail drains quickly
_SIZES = [1024, 1024, 1024, 1024, 2048, 2048] + [4096] * 27 + \
    [1024, 1024, 668]
assert sum(_SIZES) == HALF

# x is packed in DRAM with a PAD-column gap after every BLK-column block
# (per half): DRAM-side discontinuity between consecutive descriptors keeps
# the SDMA from re-aggregating 4KB read descriptors into 16KB packets (the
# read path pipelines <=4KB descriptors at ~26 GB/s/engine vs ~16 GB/s for
# 16KB ones).
BLK = 2048
PAD = 16
HALFP = (HALF // BLK) * (BLK + PAD) + (HALF % BLK)  # padded cols per half


def _pmap(c):
    """half-local flat column -> padded column"""
    return (c // BLK) * (BLK + PAD) + (c % BLK)
CHUNKS = []
_c = 0
for _sz in _SIZES:
    CHUNKS.append((_c, _sz))
    _c += _sz



_CACHE = {}


def _segments_in(lo, hi):
    """Yield (s, e, l) sub-intervals of [lo, hi) split at region bounds."""
    for s, e, l in SEGS:
        a, b = max(s, lo), min(e, hi)
        if a < b:
            yield a, b, l


def _build_program():
    """Build and finalize the (SPMD, per-core) Bass program once."""
    import concourse.bass as bass
    import concourse.tile as tile
    import concourse.mybir as mybir
    from concourse import bacc

    f32 = mybir.dt.float32
    bf16 = mybir.dt.bfloat16

    nc = bacc.Bacc("TRN2", target_bir_lowering=False, debug=False,
                   num_devices=NCORES, dynamic_dma_scratch_size=12288)
    x = nc.declare_dram_parameter("x", [P, 2 * HALFP], bf16, isOutput=False)
    # all six W_l packed side by side -> one DMA
    wall = nc.declare_dram_parameter("wall", [P, (LMAX + 1) * B], bf16,
                                     isOutput=False)
    # packed output: rows 0-63 = half A columns, rows 64-127 = half B
    y = nc.declare_dram_parameter("y", [2 * B, HALF], bf16, isOutput=True)

    with tile.TileContext(nc) as tc:
        with (
            tc.tile_pool(name="wp", bufs=1) as wp,
            # 8 input bufs with PREFETCH=5: a read issued mid-chunk ci for
            # chunk ci+6 reuses chunk ci-2's buffer, whose matmuls are long
            # done -> the issuing engine never stalls and no
            # engine-order/PSUM-bank dependency cycle can form; the deep
            # read-ahead keeps both HWDGE queues busy continuously
            tc.tile_pool(name="inp", bufs=8) as inp,
            tc.tile_pool(name="outp", bufs=6) as outp,
            tc.tile_pool(name="psp", bufs=8, space="PSUM") as psp,
        ):
            wt = wp.tile([P, (LMAX + 1) * B], bf16, name="wt", tag="wt")
            nc.scalar.dma_start(wt[:], wall[:, :])
            w_sb = [wt[:, l * B:(l + 1) * B] for l in range(LMAX + 1)]

            # issue the input DMAs PREFETCH chunks ahead of compute so read
            # issuance never sits behind copies in engine program order
            # (HWDGE rings drain asynchronously once the instruction issues)
            PREFETCH = 5
            xts = {}

            def issue_read(ci):
                c0, csz = CHUNKS[ci]
                xa = inp.tile([P, csz], bf16, name=f"xa_{ci}", tag="xa")
                xb = inp.tile([P, csz], bf16, name=f"xb_{ci}", tag="xb")

                def rd(eng, dst, base):
                    p0 = base + _pmap(c0)
                    if csz % BLK != 0:
                        # small chunk: sits inside one padded block
                        eng.dma_start(dst[:], x[:, p0:p0 + csz])
                    else:
                        # one DMA, 3-dim AP skipping the pads: 4KB read
                        # descriptors that cannot re-aggregate
                        k = csz // BLK
                        src = x[:, p0:p0 + k * (BLK + PAD)].rearrange(
                            "p (k d) -> p k d", d=BLK + PAD)[:, :, 0:BLK]
                        eng.dma_start(
                            dst[:].rearrange("p (k d) -> p k d", d=BLK), src)

                rd(nc.sync, xa, 0)
                rd(nc.scalar, xb, HALFP)
                xts[ci] = (xa, xb)

            for ci in range(min(PREFETCH + 1, len(CHUNKS))):
                issue_read(ci)

            for ci, (c0, csz) in enumerate(CHUNKS):
                xa, xb = xts.pop(ci)
                ot = outp.tile([2 * B, csz], bf16, name=f"ot_{ci}", tag="ot")
                for ti, k0 in enumerate(range(0, csz, NT)):
                    n = min(NT, csz - k0)
                    ps = psp.tile([2 * B, n], f32, name=f"ps_{ci}_{k0}",
                                  tag="ps")
                    for sa, sb, l in _segments_in(c0 + k0, c0 + k0 + n):
                        ra, rb = sa - c0, sb - c0
                        nc.tensor.matmul(ps[0:B, ra - k0:rb - k0],
                                         lhsT=w_sb[l],
                                         rhs=xa[:, ra:rb],
                                         start=True, stop=True)
                    for sa, sb, l in _segments_in(HALF + c0 + k0,
                                                  HALF + c0 + k0 + n):
                        ra, rb = sa - HALF - c0, sb - HALF - c0
                        nc.tensor.matmul(ps[B:2 * B, ra - k0:rb - k0],
                                         lhsT=w_sb[l],
                                         rhs=xb[:, ra:rb],
                                         start=True, stop=True)
                    # DVE takes 2/3 of the downcast copies, ACT 1/3
                    # (ACT also issues the xb reads)
                    if ti % 3 != 2:
                        nc.vector.tensor_copy(ot[:, k0:k0 + n], ps[:])
                    else:
                        nc.scalar.copy(ot[:, k0:k0 + n], ps[:])
                    # interleave next reads mid-compute so their issue
                    # point sits between copies, not after them
                    if ti == 1 and ci + PREFETCH + 1 < len(CHUNKS):
                        issue_read(ci + PREFETCH + 1)
                # per-chunk write: SWDGE writes run at line rate at any
                # descriptor size; finer cadence keeps the write stream
                # continuously overlapped with reads
                nc.gpsimd.dma_start(y[:, c0:c0 + csz], ot[:])

    nc.finalize()
    return nc


def _get_program():
    if "nc" not in _CACHE:
        _CACHE["nc"] = _build_program()
    return _CACHE["nc"]


def _register_ntff_hook():
    """antenv.axon_hooks is absent in this image; the .so supports NTFF
    profiling — install the shim so run_bass_kernel_spmd(trace=True) works."""
    import sys, types
    try:
        from antenv.axon_hooks import get_axon_ntff_profile_hook  # noqa: F401
        return
    except ImportError:
        pass
    import antenv
    from trn_agent_boot.trn_boot import _ntff_profile_via_ctypes
    mod = types.ModuleType("antenv.axon_hooks")
    mod._hook = _ntff_profile_via_ctypes('/opt/axon/libaxon_pjrt.so')
    mod.get_axon_ntff_profile_hook = lambda: mod._hook
    mod.set_axon_ntff_profile_hook = lambda h: setattr(mod, '_hook', h)
    sys.modules["antenv.axon_hooks"] = mod
    antenv.axon_hooks = mod


LAST_EXEC_TIME_NS = None
LAST_MEAN_EXEC_TIME_NS = None


def kernel(trace=False, trace_all_cores=False, **inputs):
    global LAST_EXEC_TIME_NS, LAST_MEAN_EXEC_TIME_NS
    from concourse.bass_utils import run_bass_kernel_spmd

    # ---- host-side: shard, transpose to [P, S], pack flat, cast bf16 ----
    wall = np.concatenate(
        [np.asarray(inputs[f"W_l{l}"]) for l in range(LMAX + 1)],
        axis=1).astype(BF16)
    flats = [np.empty((P, STOT), dtype=BF16) for _ in range(NCORES)]
    off = 0
    for g, l, cols in REGIONS:
        src = inputs[f"grads_l{l}"] if g else inputs[f"values_l{l}"]
        src = np.asarray(src)
        ns = NGV if g else NV
        for i in range(NCORES):
            blk = src[i * ns:(i + 1) * ns].reshape(cols, P).astype(BF16)
            flats[i][:, off:off + cols] = blk.T
        off += cols
    # scatter each half's 2048-col blocks into the PAD-gapped DRAM layout
    nfull = HALF // BLK
    in_maps = []
    for i in range(NCORES):
        xp = np.zeros((P, 2 * HALFP), dtype=BF16)
        for h in range(2):
            fl = flats[i][:, h * HALF:(h + 1) * HALF]
            dst = xp[:, h * HALFP:(h + 1) * HALFP]
            dst[:, :nfull * (BLK + PAD)].reshape(P, nfull, BLK + PAD)[
                :, :, :BLK] = fl[:, :nfull * BLK].reshape(P, nfull, BLK)
            dst[:, nfull * (BLK + PAD):] = fl[:, nfull * BLK:]
        in_maps.append({"x": xp, "wall": wall})

    nc = _get_program()
    kwargs = {}
    if trace:
        _register_ntff_hook()
        kwargs["trace"] = True
        if trace_all_cores:
            kwargs["trace_cores"] = list(range(NCORES))
    res = run_bass_kernel_spmd(nc, in_maps, list(range(NCORES)), **kwargs)
    LAST_EXEC_TIME_NS = res.exec_time_ns
    LAST_MEAN_EXEC_TIME_NS = res.mean_exec_time_ns

    # ---- gather: split flat intervals across the packed halves, upcast ----
    outs = [res.results[i]["y"] for i in range(NCORES)]
    total_rows = NCORES * STOT
    final = np.empty((total_rows, B), dtype=np.float32)
    row = 0
    off = 0
    for g, l, cols in REGIONS:
        for i in range(NCORES):
            s, e = off, off + cols
            r = row
            if s < HALF:
                e0 = min(e, HALF)
                final[r:r + (e0 - s)] = outs[i][0:B, s:e0].T
                r += e0 - s
                s = e0
            if s < e:
                final[r:r + (e - s)] = outs[i][B:2 * B, s - HALF:e - HALF].T
            row += cols
        off += cols
    return final
